# revision 68
# baseline (speedup 1.0000x reference)
"""Trainium2 Bass kernel for nn_BestNet_46196668236142 (LRU block).

Pipeline per token: LN1 -> leaky -> complex diagonal recurrence over T
-> y = Re(C h) + D z -> leaky(LN2) -> MLP -> LN3 -> +skip.

Strategy:
- Data-parallel: shard B=32 across 8 cores (4 samples/core).
- The complex recurrence h_t = lam*h_{t-1} + u_t (lam = r*e^{i th}) is
  decoupled into two REAL per-channel scans via polar rotation:
      g_t = e^{-i th t} h_t   =>   g_t = r * g_{t-1} + e^{-i th t} u_t
  which maps onto the HW tensor_tensor_scan (op0=mult, op1=add) along
  the free (time) axis, n on partitions. Pre/post rotations use
  host-precomputed cos/sin tables packed as [cos|cos],[sin|sin] pairs
  so each rotation half is one wide elementwise op.
- Chunked over time (CT=512) with a tiny [P,1] carry between chunks.
- Loop order c-outer / b-inner: adjacent emitted iterations belong to
  independent samples, keeping every engine queue fed.
- Elementwise work is spread across Vector/Scalar/GpSimd via the EW
  assignment table; LN2/LN3 stats ride scalar-engine accumulators so
  the single-buffered PSUM tiles are freed after one instruction.
- All activations use table-stable functions (Identity/Copy/Sqrt/Prelu
  live in one HW table) to avoid 1.3us ACT_TABLE_LOADs.
"""

import os
import sys

import numpy as np

for _p in ("/opt/trn_rl_repo", "/root/.axon_site/_ro/trn_rl_repo"):
    if os.path.isdir(_p) and _p not in sys.path:
        sys.path.insert(0, _p)

import concourse.bass as bass
import concourse.mybir as mybir
from concourse import bacc, masks, tile
from concourse.bass_utils import run_bass_kernel_spmd

B, T, D, N = 32, 4096, 256, 256
NCORES = 8
BS = B // NCORES            # batches per core
CT = 512                    # time chunk
NSUB = CT // 128            # token subtiles per chunk
NCH = T // CT               # chunks per batch
EPS = 1e-5
SLOPE = 0.01
F32 = mybir.dt.float32
AO = mybir.AluOpType
AF = mybir.ActivationFunctionType

# Engine assignment for elementwise sites (tuned via trace iterations).
EW = {
    "prerot": "vector",    # mC/mD [128,1024], reads PSUM
    "wcomb": "vector",     # wr/wi [128,512] SBUF
    "scan": "vector",
    "carry": "vector",
    "postA": "gpsimd",     # mA halves [128,512] SBUF
    "postB": "gpsimd",     # mB halves [128,512] SBUF
    "hcomb": "gpsimd",     # hre/him [128,512] SBUF
    "zl": "scalar",        # leaky of z [128,256]; scalar=Prelu, v=stt
    "skip": "gpsimd",      # final skip add [128,256] SBUF
    "copy1": "scalar",     # stage1 transpose evac [128,128] PSUM->SBUF
    "copy5": "scalar",     # stage5 transpose evac
}

_PROG_CACHE = {}


def _build_program(flags):
    """flags = (g1, g2, g3, bias, mask) booleans for the general path."""
    g1, g2, g3, use_bias, use_mask = flags
    nc = bacc.Bacc(None, target_bir_lowering=False)

    def eng(site):
        return getattr(nc, EW[site])

    def copy_op(site, out_ap, in_ap):
        if EW[site] == "scalar":
            nc.scalar.copy(out_ap, in_ap)
        else:
            eng(site).tensor_copy(out_ap, in_ap)

    x_d = nc.declare_dram_parameter("x", [BS, T, D], F32, isOutput=False)
    q0r_d = nc.declare_dram_parameter("q0r", [BS, N], F32, isOutput=False)
    q0i_d = nc.declare_dram_parameter("q0i", [BS, N], F32, isOutput=False)
    cos2_d = nc.declare_dram_parameter("cos2", [N, 2 * CT], F32, isOutput=False)
    sin2_d = nc.declare_dram_parameter("sin2", [N, 2 * CT], F32, isOutput=False)
    rbc_d = nc.declare_dram_parameter("rbc", [N, CT], F32, isOutput=False)
    ecl_d = nc.declare_dram_parameter("ecl", [N, 1], F32, isOutput=False)
    esl_d = nc.declare_dram_parameter("esl", [N, 1], F32, isOutput=False)
    brt_d = nc.declare_dram_parameter("BrT", [D, N], F32, isOutput=False)
    bit_d = nc.declare_dram_parameter("BiT", [D, N], F32, isOutput=False)
    crt_d = nc.declare_dram_parameter("CrT", [N, D], F32, isOutput=False)
    cin_d = nc.declare_dram_parameter("CiTn", [N, D], F32, isOutput=False)
    dt_d = nc.declare_dram_parameter("DT", [D, N], F32, isOutput=False)
    mt_d = nc.declare_dram_parameter("MT", [N, D], F32, isOutput=False)
    out_d = nc.declare_dram_parameter("out", [BS, T, D], F32, isOutput=True)

    if use_mask:
        d0_d = nc.declare_dram_parameter("d0tab", [BS, N, T], F32, isOutput=False)
    gb_params = {}
    for name, on in (("g1", g1), ("b1", g1), ("g2", g2), ("b2", g2),
                     ("g3", g3), ("b3", g3), ("mb", use_bias)):
        if on:
            gb_params[name] = nc.declare_dram_parameter(name + "bc", [128, D], F32)

    from contextlib import ExitStack

    with tile.TileContext(nc) as tc, ExitStack() as ctx:
        cpool = ctx.enter_context(tc.tile_pool(name="consts", bufs=1))

        _cn = [0]

        def cload(dram, shape):
            _cn[0] += 1
            t = cpool.tile(shape, F32, name=f"const{_cn[0]}",
                           tag=f"const{_cn[0]}")
            nc.sync.dma_start(t[:], dram)
            return t

        cos2 = [cload(cos2_d[p * 128:(p + 1) * 128, :], [128, 2 * CT])
                for p in range(2)]
        sin2 = [cload(sin2_d[p * 128:(p + 1) * 128, :], [128, 2 * CT])
                for p in range(2)]
        rbc = [cload(rbc_d[p * 128:(p + 1) * 128, :], [128, CT]) for p in range(2)]
        ecl = [cload(ecl_d[p * 128:(p + 1) * 128, :], [128, 1]) for p in range(2)]
        esl = [cload(esl_d[p * 128:(p + 1) * 128, :], [128, 1]) for p in range(2)]
        brt = [cload(brt_d[k * 128:(k + 1) * 128, :], [128, N]) for k in range(2)]
        bit = [cload(bit_d[k * 128:(k + 1) * 128, :], [128, N]) for k in range(2)]
        crt = [cload(crt_d[p * 128:(p + 1) * 128, :], [128, D]) for p in range(2)]
        cin = [cload(cin_d[p * 128:(p + 1) * 128, :], [128, D]) for p in range(2)]
        dts = [cload(dt_d[k * 128:(k + 1) * 128, :], [128, N]) for k in range(2)]
        mts = [cload(mt_d[p * 128:(p + 1) * 128, :], [128, D]) for p in range(2)]
        gbt = {k: cload(v[:, :], [128, D]) for k, v in gb_params.items()}
        ident = cpool.tile([128, 128], F32)
        masks.make_identity(nc, ident[:])
        epst = cpool.tile([128, 1], F32)
        nc.gpsimd.memset(epst[:], EPS)

        xin = ctx.enter_context(tc.tile_pool(name="xin", bufs=8))
        zskip = ctx.enter_context(tc.tile_pool(name="zskip", bufs=12))
        zlp = ctx.enter_context(tc.tile_pool(name="zl", bufs=6))
        ztp = ctx.enter_context(tc.tile_pool(name="zt", bufs=4))
        stat = ctx.enter_context(tc.tile_pool(name="stat", bufs=20))
        gip = ctx.enter_context(tc.tile_pool(name="gi", bufs=24))
        big = ctx.enter_context(tc.tile_pool(name="big", bufs=2))
        wp = ctx.enter_context(tc.tile_pool(name="w", bufs=4))
        g2p = ctx.enter_context(tc.tile_pool(name="g2", bufs=3))
        h2p = ctx.enter_context(tc.tile_pool(name="h2", bufs=5))
        yl2p = ctx.enter_context(tc.tile_pool(name="yl2", bufs=5))
        y2tp = ctx.enter_context(tc.tile_pool(name="y2t", bufs=2))
        yop = ctx.enter_context(tc.tile_pool(name="yo", bufs=5))
        if use_mask:
            d0p = ctx.enter_context(tc.tile_pool(name="d0p", bufs=4))
        # PSUM banks: 4 (u2 ring 2) + 2 (ptr [128,1024]) + 2 (pyx ring) = 8
        pu = ctx.enter_context(
            tc.tile_pool(name="pu", bufs=2, space=bass.MemorySpace.PSUM))
        ptr = ctx.enter_context(
            tc.tile_pool(name="ptr", bufs=1, space=bass.MemorySpace.PSUM))
        pyx = ctx.enter_context(
            tc.tile_pool(name="pyx", bufs=2, space=bass.MemorySpace.PSUM))

        def ln_stats_v(src_ap):
            """bn_stats path -> (rstd, nmr) [128,1]; V + one scalar sqrt."""
            st6 = stat.tile([128, 6], F32)
            nc.vector.bn_stats(st6[:], src_ap)
            mv = stat.tile([128, 2], F32)
            nc.vector.bn_aggr(mv[:], st6[:])
            std = stat.tile([128, 1], F32)
            nc.scalar.activation(std[:], mv[:, 1:2], AF.Sqrt, bias=epst[:])
            rstd = stat.tile([128, 1], F32)
            nc.vector.reciprocal(rstd[:], std[:])
            nmr = stat.tile([128, 1], F32)
            nc.vector.tensor_scalar(
                nmr[:], mv[:, 0:1], rstd[:], -1.0, op0=AO.mult, op1=AO.mult)
            return rstd, nmr

        # persistent scan carries, one [128,1] per (b, p, comp)
        ginit = {}
        for b in range(BS):
            for p in range(2):
                for comp, src in ((0, q0r_d), (1, q0i_d)):
                    t = gip.tile([128, 1], F32, name="giq")
                    nc.sync.dma_start(t[:], src[b, p * 128:(p + 1) * 128])
                    ginit[(b, p, comp)] = t

        def emit_a(c, b):
            """Stage 1+2: load, LN1 (batched small-ops), leaky, transpose,
            B-projection into PSUM."""
            t0 = c * CT
            if True:
                # ---- stage 1: load, LN1, leaky, transpose ----
                zt = ztp.tile([128, 2 * CT], F32, name="zt")
                zsk = []
                ptb = ptr.tile([128, 2 * CT], F32, name="pt", tag="pt")
                for s in range(NSUB):
                    xt = xin.tile([128, D], F32)
                    nc.sync.dma_start(
                        xt[:], x_d[b, t0 + s * 128:t0 + (s + 1) * 128, :])
                    rstd, nmr = ln_stats_v(xt[:])
                    z = zskip.tile([128, D], F32)
                    nc.scalar.activation(
                        z[:], xt[:], AF.Identity, bias=nmr[:], scale=rstd[:])
                    if g1:
                        nc.vector.tensor_mul(z[:], z[:], gbt["g1"][:])
                        nc.vector.tensor_add(z[:], z[:], gbt["b1"][:])
                    zsk.append(z)
                    zl = zlp.tile([128, D], F32)
                    nc.scalar.activation(zl[:], z[:], AF.Prelu, alpha=SLOPE)
                    for k in range(2):
                        nc.tensor.transpose(
                            ptb[:, k * CT + s * 128:k * CT + (s + 1) * 128],
                            zl[:, k * 128:(k + 1) * 128], ident[:])
                # 2 wide evacuations instead of 8 narrow ones
                for k in range(2):
                    copy_op("copy1", zt[:, k * CT:(k + 1) * CT],
                            ptb[:, k * CT:(k + 1) * CT])
                # ---- stage 2: B projection -> u2[p] = [u_re | u_im] PSUM ----
                u2 = []
                for p in range(2):
                    ut = pu.tile([128, 2 * CT], F32)
                    for comp, bt in ((0, brt), (1, bit)):
                        for k in range(2):
                            nc.tensor.matmul(
                                ut[:, comp * CT:(comp + 1) * CT],
                                bt[k][:, p * 128:(p + 1) * 128],
                                zt[:, k * CT:(k + 1) * CT],
                                start=(k == 0), stop=(k == 1))
                    u2.append(ut)
            return dict(zt=zt, zsk=zsk, u2=u2)

        def emit_b(c, b, st):
            """Stage 3+4: pre-rotation, scans, carry, post-rotation."""
            t0 = c * CT
            u2 = st["u2"]
            if True:
                # ---- stage 3: pre-rotation + scans + carry ----
                # both prerots first (frees the u2 PSUM ring for the next
                # iteration's B-projection as early as possible)
                wcs = []
                for p in range(2):
                    mc = big.tile([128, 2 * CT], F32)
                    eng("prerot").tensor_mul(mc[:], cos2[p][:], u2[p][:])
                    md = big.tile([128, 2 * CT], F32)
                    eng("prerot").tensor_mul(md[:], sin2[p][:], u2[p][:])
                    wcs.append((mc, md))
                gts = []
                for p in range(2):
                    if use_mask:
                        d0 = d0p.tile([128, CT], F32)
                        nc.sync.dma_start(
                            d0[:], d0_d[b, p * 128:(p + 1) * 128, t0:t0 + CT])
                        d0ap = d0[:]
                    else:
                        d0ap = rbc[p][:]
                    mc, md = wcs[p]
                    wr = wp.tile([128, CT], F32)
                    eng("wcomb").tensor_add(wr[:], mc[:, :CT], md[:, CT:])
                    wi = wp.tile([128, CT], F32)
                    eng("wcomb").tensor_sub(wi[:], mc[:, CT:], md[:, :CT])
                    gr = g2p.tile([128, CT], F32, name="gr")
                    eng("scan").tensor_tensor_scan(
                        gr[:], d0ap, wr[:], ginit[(b, p, 0)][:],
                        op0=AO.mult, op1=AO.add)
                    gi = g2p.tile([128, CT], F32, name="gi")
                    eng("scan").tensor_tensor_scan(
                        gi[:], d0ap, wi[:], ginit[(b, p, 1)][:],
                        op0=AO.mult, op1=AO.add)
                    gts.append((gr, gi))
                    if c + 1 < NCH:
                        grl = gr[:, CT - 1:CT]
                        gil = gi[:, CT - 1:CT]
                        t5 = gip.tile([128, 1], F32)
                        eng("carry").tensor_scalar_mul(t5[:], gil, esl[p][:])
                        ngr = gip.tile([128, 1], F32)
                        eng("carry").scalar_tensor_tensor(
                            ngr[:], grl, ecl[p][:], t5[:],
                            op0=AO.mult, op1=AO.subtract)
                        t6 = gip.tile([128, 1], F32)
                        eng("carry").tensor_scalar_mul(t6[:], grl, esl[p][:])
                        ngi = gip.tile([128, 1], F32)
                        eng("carry").scalar_tensor_tensor(
                            ngi[:], gil, ecl[p][:], t6[:],
                            op0=AO.mult, op1=AO.add)
                        ginit[(b, p, 0)], ginit[(b, p, 1)] = ngr, ngi
                # ---- stage 4: post-rotation -> h2[p] = [h_re | h_im] ----
                h2 = []
                for p in range(2):
                    gr, gi = gts[p]
                    cosj = cos2[p][:, :CT]
                    sinj = sin2[p][:, :CT]
                    ma = big.tile([128, 2 * CT], F32)
                    eng("postA").tensor_mul(ma[:, :CT], cosj, gr[:])
                    eng("postA").tensor_mul(ma[:, CT:], cosj, gi[:])
                    mb = big.tile([128, 2 * CT], F32)
                    eng("postB").tensor_mul(mb[:, :CT], sinj, gr[:])
                    eng("postB").tensor_mul(mb[:, CT:], sinj, gi[:])
                    ht = h2p.tile([128, 2 * CT], F32, name="h2")
                    eng("hcomb").tensor_sub(ht[:, :CT], ma[:, :CT], mb[:, CT:])
                    eng("hcomb").tensor_add(ht[:, CT:], ma[:, CT:], mb[:, :CT])
                    h2.append(ht)
            st["h2"] = h2
            return st

        def emit_c(c, b, st):
            """Stages 5-6: C/D projection, LN2, MLP, LN3, skip, store."""
            t0 = c * CT
            zt, h2, zsk = st["zt"], st["h2"], st["zsk"]
            if True:
                # ---- stage 5: C/D projection + LN2 + leaky + transpose ----
                y2t = y2tp.tile([128, 2 * CT], F32, name="y2t")
                yl2s = []
                for s in range(NSUB):
                    pt1 = pyx.tile([128, D], F32, name="pyx", tag="pyx")[:]
                    mms = []
                    for k in range(2):
                        mms.append((zt[:, k * CT + s * 128:k * CT + (s + 1) * 128],
                                    dts[k][:]))
                    for p in range(2):
                        mms.append((h2[p][:, s * 128:(s + 1) * 128], crt[p][:]))
                        mms.append((h2[p][:, CT + s * 128:CT + (s + 1) * 128],
                                    cin[p][:]))
                    for i, (lhs, rhs) in enumerate(mms):
                        nc.tensor.matmul(pt1, lhs, rhs, start=(i == 0),
                                         stop=(i == len(mms) - 1))
                    rstd, nmr = ln_stats_v(pt1)
                    yl2 = yl2p.tile([128, D], F32)
                    if g2:
                        nc.scalar.activation(
                            yl2[:], pt1, AF.Identity, bias=nmr[:], scale=rstd[:])
                        nc.vector.tensor_mul(yl2[:], yl2[:], gbt["g2"][:])
                        nc.vector.tensor_add(yl2[:], yl2[:], gbt["b2"][:])
                        nc.scalar.activation(yl2[:], yl2[:], AF.Prelu, alpha=SLOPE)
                    else:
                        nc.scalar.activation(
                            yl2[:], pt1, AF.Prelu, bias=nmr[:], scale=rstd[:],
                            alpha=SLOPE)
                    yl2s.append(yl2)
                ptb5 = ptr.tile([128, 2 * CT], F32, name="pt", tag="pt")
                for s in range(NSUB):
                    for p in range(2):
                        nc.tensor.transpose(
                            ptb5[:, p * CT + s * 128:p * CT + (s + 1) * 128],
                            yl2s[s][:, p * 128:(p + 1) * 128], ident[:])
                for p in range(2):
                    copy_op("copy5", y2t[:, p * CT:(p + 1) * CT],
                            ptb5[:, p * CT:(p + 1) * CT])
                # ---- stage 6: MLP + LN3 + skip + store ----
                for s in range(NSUB):
                    pt3 = pyx.tile([128, D], F32, name="pyx", tag="pyx")[:]
                    for p in range(2):
                        nc.tensor.matmul(
                            pt3, y2t[:, p * CT + s * 128:p * CT + (s + 1) * 128],
                            mts[p][:], start=(p == 0), stop=(p == 1))
                    if use_bias:
                        nc.vector.tensor_add(pt3, pt3, gbt["mb"][:])
                    rstd, nmr = ln_stats_v(pt3)
                    yo = yop.tile([128, D], F32)
                    nc.scalar.activation(
                        yo[:], pt3, AF.Identity, bias=nmr[:], scale=rstd[:])
                    if g3:
                        nc.vector.tensor_mul(yo[:], yo[:], gbt["g3"][:])
                        nc.vector.tensor_add(yo[:], yo[:], gbt["b3"][:])
                    eng("skip").tensor_add(yo[:], yo[:], zsk[s][:])
                    nc.sync.dma_start(
                        out_d[b, t0 + s * 128:t0 + (s + 1) * 128, :], yo[:])

        # 3-phase software pipeline: per step k emit A(k), B(k-1), C(k-2)
        # so each engine queue interleaves three iterations and the PE
        # always has independent matmul work while the V/G rotation/scan
        # chains of older iterations complete.
        iters = [(c, b) for c in range(NCH) for b in range(BS)]
        n = len(iters)
        states = {}
        for k in range(n + 2):
            if 1 <= k and k - 1 < n:
                emit_b(*iters[k - 1], states[k - 1])
            if k < n:
                states[k] = emit_a(*iters[k])
            if 2 <= k:
                emit_c(*iters[k - 2], states.pop(k - 2))
    nc.compile()
    return nc


def _prep_host(inputs):
    """Host-side precompute: tables, folded weights, per-core input maps."""
    x = np.asarray(inputs["x"], np.float32)
    done = np.asarray(inputs["done"])
    h0r = np.asarray(inputs["h0_re"], np.float32)
    h0i = np.asarray(inputs["h0_im"], np.float32)
    nu = np.asarray(inputs["nu_log"], np.float64)
    th_log = np.asarray(inputs["theta_log"], np.float64)
    gl = np.asarray(inputs["gamma_log"], np.float64)

    r = np.exp(-np.exp(nu))                     # |lambda|, [N]
    theta = np.exp(th_log)                      # [N]
    gamma = np.exp(gl)

    j = np.arange(CT, dtype=np.float64)
    ang = theta[:, None] * j[None, :]           # [N, CT]
    cosj = np.cos(ang).astype(np.float32)
    sinj = np.sin(ang).astype(np.float32)
    cos2 = np.concatenate([cosj, cosj], axis=1)  # [N, 2CT]
    sin2 = np.concatenate([sinj, sinj], axis=1)
    rbc = np.repeat(r.astype(np.float32)[:, None], CT, axis=1)
    angL = theta * CT
    ecl = np.cos(angL).astype(np.float32)[:, None]
    esl = np.sin(angL).astype(np.float32)[:, None]

    # q0 = e^{i theta} * h0  per (b, n)
    c1, s1 = np.cos(theta), np.sin(theta)
    q0r = (c1[None, :] * h0r - s1[None, :] * h0i).astype(np.float32)
    q0i = (c1[None, :] * h0i + s1[None, :] * h0r).astype(np.float32)

    brt = np.ascontiguousarray(
        (np.asarray(inputs["B_re"], np.float64) * gamma[:, None]).T
    ).astype(np.float32)
    bit = np.ascontiguousarray(
        (np.asarray(inputs["B_im"], np.float64) * gamma[:, None]).T
    ).astype(np.float32)
    crt = np.ascontiguousarray(np.asarray(inputs["C_re"], np.float32).T)
    cin = np.ascontiguousarray(-np.asarray(inputs["C_im"], np.float32).T)
    dt = np.ascontiguousarray(np.asarray(inputs["D_mat"], np.float32).T)
    mt = np.ascontiguousarray(np.asarray(inputs["mlp_w"], np.float32).T)

    g1v = np.asarray(inputs["ln1_g"], np.float32)
    b1v = np.asarray(inputs["ln1_b"], np.float32)
    g2v = np.asarray(inputs["ln2_g"], np.float32)
    b2v = np.asarray(inputs["ln2_b"], np.float32)
    g3v = np.asarray(inputs["ln3_g"], np.float32)
    b3v = np.asarray(inputs["ln3_b"], np.float32)
    mbv = np.asarray(inputs["mlp_b"], np.float32)

    g1 = not (np.all(g1v == 1) and np.all(b1v == 0))
    g2 = not (np.all(g2v == 1) and np.all(b2v == 0))
    g3 = not (np.all(g3v == 1) and np.all(b3v == 0))
    use_bias = bool(np.any(mbv != 0))
    use_mask = bool(np.any(done))
    flags = (g1, g2, g3, use_bias, use_mask)

    shared = dict(cos2=cos2, sin2=sin2, rbc=rbc, ecl=ecl, esl=esl,
                  BrT=brt, BiT=bit, CrT=crt, CiTn=cin, DT=dt, MT=mt)

    def bc(v):
        return np.ascontiguousarray(np.broadcast_to(v[None, :], (128, D))
                                    ).astype(np.float32)
    if g1:
        shared["g1bc"], shared["b1bc"] = bc(g1v), bc(b1v)
    if g2:
        shared["g2bc"], shared["b2bc"] = bc(g2v), bc(b2v)
    if g3:
        shared["g3bc"], shared["b3bc"] = bc(g3v), bc(b3v)
    if use_bias:
        shared["mbbc"] = bc(mbv)

    in_maps = []
    for core in range(NCORES):
        sl = slice(core * BS, (core + 1) * BS)
        m = dict(shared)
        m["x"] = np.ascontiguousarray(x[sl])
        m["q0r"] = np.ascontiguousarray(q0r[sl])
        m["q0i"] = np.ascontiguousarray(q0i[sl])
        if use_mask:
            mask = 1.0 - done[sl].astype(np.float32)       # [BS, T]
            d0 = (rbc[None, :, 0:1] * mask[:, None, :])    # [BS, N, T]
            m["d0tab"] = np.ascontiguousarray(d0.astype(np.float32))
        in_maps.append(m)
    return flags, in_maps


def _get_program(flags):
    if flags not in _PROG_CACHE:
        _PROG_CACHE[flags] = _build_program(flags)
    return _PROG_CACHE[flags]


def run(inputs, trace=False, **kw):
    flags, in_maps = _prep_host(inputs)
    nc = _get_program(flags)
    res = run_bass_kernel_spmd(nc, in_maps, list(range(NCORES)),
                               trace=trace, **kw)
    out = np.concatenate([res.results[i]["out"] for i in range(NCORES)], axis=0)
    return out, res


def kernel(**inputs):
    out, _ = run(inputs, trace=False)
    return out


# revision 72
# speedup vs baseline: 1.0329x; 1.0329x over previous
"""Trainium2 Bass kernel for nn_BestNet_46196668236142 (LRU block).

Pipeline per token: LN1 -> leaky -> complex diagonal recurrence over T
-> y = Re(C h) + D z -> leaky(LN2) -> MLP -> LN3 -> +skip.

Strategy:
- Data-parallel: shard B=32 across 8 cores (4 samples/core).
- The complex recurrence h_t = lam*h_{t-1} + u_t (lam = r*e^{i th}) is
  decoupled into two REAL per-channel scans via polar rotation:
      g_t = e^{-i th t} h_t   =>   g_t = r * g_{t-1} + e^{-i th t} u_t
  which maps onto the HW tensor_tensor_scan (op0=mult, op1=add) along
  the free (time) axis, n on partitions. Pre/post rotations use
  host-precomputed cos/sin tables packed as [cos|cos],[sin|sin] pairs
  so each rotation half is one wide elementwise op.
- Chunked over time (CT=512) with a tiny [P,1] carry between chunks.
- Loop order c-outer / b-inner: adjacent emitted iterations belong to
  independent samples, keeping every engine queue fed.
- Elementwise work is spread across Vector/Scalar/GpSimd via the EW
  assignment table; LN2/LN3 stats ride scalar-engine accumulators so
  the single-buffered PSUM tiles are freed after one instruction.
- All activations use table-stable functions (Identity/Copy/Sqrt/Prelu
  live in one HW table) to avoid 1.3us ACT_TABLE_LOADs.
"""

import os
import sys

import numpy as np

for _p in ("/opt/trn_rl_repo", "/root/.axon_site/_ro/trn_rl_repo"):
    if os.path.isdir(_p) and _p not in sys.path:
        sys.path.insert(0, _p)

import concourse.bass as bass
import concourse.mybir as mybir
from concourse import bacc, masks, tile
from concourse.bass_utils import run_bass_kernel_spmd

B, T, D, N = 32, 4096, 256, 256
NCORES = 8
BS = B // NCORES            # batches per core
CT = 512                    # time chunk
NSUB = CT // 128            # token subtiles per chunk
NCH = T // CT               # chunks per batch
EPS = 1e-5
SLOPE = 0.01
F32 = mybir.dt.float32
AO = mybir.AluOpType
AF = mybir.ActivationFunctionType

# Engine assignment for elementwise sites (tuned via trace iterations).
EW = {
    "prerot": "vector",    # mC/mD [128,1024], reads PSUM
    "wcomb": "gpsimd",     # wr/wi [128,512] SBUF
    "scan": "vector",
    "carry": "vector",
    "postA": "gpsimd",     # mA halves [128,512] SBUF
    "postB": "gpsimd",     # mB halves [128,512] SBUF
    "hcomb": "gpsimd",     # hre/him [128,512] SBUF
    "zl": "scalar",        # leaky of z [128,256]; scalar=Prelu, v=stt
    "skip": "gpsimd",      # final skip add [128,256] SBUF
    "copy1": "scalar",     # stage1 transpose evac [128,128] PSUM->SBUF
    "copy5": "scalar",     # stage5 transpose evac
}

_PROG_CACHE = {}


def _build_program(flags):
    """flags = (g1, g2, g3, bias, mask) booleans for the general path."""
    g1, g2, g3, use_bias, use_mask = flags
    nc = bacc.Bacc(None, target_bir_lowering=False)

    def eng(site):
        return getattr(nc, EW[site])

    def copy_op(site, out_ap, in_ap):
        if EW[site] == "scalar":
            nc.scalar.copy(out_ap, in_ap)
        else:
            eng(site).tensor_copy(out_ap, in_ap)

    x_d = nc.declare_dram_parameter("x", [BS, T, D], F32, isOutput=False)
    q0r_d = nc.declare_dram_parameter("q0r", [BS, N], F32, isOutput=False)
    q0i_d = nc.declare_dram_parameter("q0i", [BS, N], F32, isOutput=False)
    cos2_d = nc.declare_dram_parameter("cos2", [N, 2 * CT], F32, isOutput=False)
    sin2_d = nc.declare_dram_parameter("sin2", [N, 2 * CT], F32, isOutput=False)
    rbc_d = nc.declare_dram_parameter("rbc", [N, CT], F32, isOutput=False)
    ecl_d = nc.declare_dram_parameter("ecl", [N, 1], F32, isOutput=False)
    esl_d = nc.declare_dram_parameter("esl", [N, 1], F32, isOutput=False)
    brt_d = nc.declare_dram_parameter("BrT", [D, N], F32, isOutput=False)
    bit_d = nc.declare_dram_parameter("BiT", [D, N], F32, isOutput=False)
    crt_d = nc.declare_dram_parameter("CrT", [N, D], F32, isOutput=False)
    cin_d = nc.declare_dram_parameter("CiTn", [N, D], F32, isOutput=False)
    dt_d = nc.declare_dram_parameter("DT", [D, N], F32, isOutput=False)
    mt_d = nc.declare_dram_parameter("MT", [N, D], F32, isOutput=False)
    out_d = nc.declare_dram_parameter("out", [BS, T, D], F32, isOutput=True)

    if use_mask:
        d0_d = nc.declare_dram_parameter("d0tab", [BS, N, T], F32, isOutput=False)
    gb_params = {}
    for name, on in (("g1", g1), ("b1", g1), ("g2", g2), ("b2", g2),
                     ("g3", g3), ("b3", g3), ("mb", use_bias)):
        if on:
            gb_params[name] = nc.declare_dram_parameter(name + "bc", [128, D], F32)

    from contextlib import ExitStack

    with tile.TileContext(nc) as tc, ExitStack() as ctx:
        cpool = ctx.enter_context(tc.tile_pool(name="consts", bufs=1))

        _cn = [0]

        def cload(dram, shape):
            _cn[0] += 1
            t = cpool.tile(shape, F32, name=f"const{_cn[0]}",
                           tag=f"const{_cn[0]}")
            nc.sync.dma_start(t[:], dram)
            return t

        cos2 = [cload(cos2_d[p * 128:(p + 1) * 128, :], [128, 2 * CT])
                for p in range(2)]
        sin2 = [cload(sin2_d[p * 128:(p + 1) * 128, :], [128, 2 * CT])
                for p in range(2)]
        rbc = [cload(rbc_d[p * 128:(p + 1) * 128, :], [128, CT]) for p in range(2)]
        ecl = [cload(ecl_d[p * 128:(p + 1) * 128, :], [128, 1]) for p in range(2)]
        esl = [cload(esl_d[p * 128:(p + 1) * 128, :], [128, 1]) for p in range(2)]
        brt = [cload(brt_d[k * 128:(k + 1) * 128, :], [128, N]) for k in range(2)]
        bit = [cload(bit_d[k * 128:(k + 1) * 128, :], [128, N]) for k in range(2)]
        crt = [cload(crt_d[p * 128:(p + 1) * 128, :], [128, D]) for p in range(2)]
        cin = [cload(cin_d[p * 128:(p + 1) * 128, :], [128, D]) for p in range(2)]
        dts = [cload(dt_d[k * 128:(k + 1) * 128, :], [128, N]) for k in range(2)]
        mts = [cload(mt_d[p * 128:(p + 1) * 128, :], [128, D]) for p in range(2)]
        gbt = {k: cload(v[:, :], [128, D]) for k, v in gb_params.items()}
        ident = cpool.tile([128, 128], F32)
        masks.make_identity(nc, ident[:])
        epst = cpool.tile([128, 1], F32)
        nc.gpsimd.memset(epst[:], EPS)

        xin = ctx.enter_context(tc.tile_pool(name="xin", bufs=8))
        zskip = ctx.enter_context(tc.tile_pool(name="zskip", bufs=12))
        zlp = ctx.enter_context(tc.tile_pool(name="zl", bufs=6))
        ztp = ctx.enter_context(tc.tile_pool(name="zt", bufs=4))
        stat = ctx.enter_context(tc.tile_pool(name="stat", bufs=20))
        gip = ctx.enter_context(tc.tile_pool(name="gi", bufs=24))
        big = ctx.enter_context(tc.tile_pool(name="big", bufs=2))
        wp = ctx.enter_context(tc.tile_pool(name="w", bufs=4))
        g2p = ctx.enter_context(tc.tile_pool(name="g2", bufs=3))
        h2p = ctx.enter_context(tc.tile_pool(name="h2", bufs=5))
        yl2p = ctx.enter_context(tc.tile_pool(name="yl2", bufs=5))
        y2tp = ctx.enter_context(tc.tile_pool(name="y2t", bufs=2))
        yop = ctx.enter_context(tc.tile_pool(name="yo", bufs=5))
        if use_mask:
            d0p = ctx.enter_context(tc.tile_pool(name="d0p", bufs=4))
        # PSUM banks: 4 (u2 ring 2) + 2 (ptr [128,1024]) + 2 (pyx ring) = 8
        pu = ctx.enter_context(
            tc.tile_pool(name="pu", bufs=2, space=bass.MemorySpace.PSUM))
        ptr = ctx.enter_context(
            tc.tile_pool(name="ptr", bufs=2, space=bass.MemorySpace.PSUM))
        pyx = ctx.enter_context(
            tc.tile_pool(name="pyx", bufs=2, space=bass.MemorySpace.PSUM))

        def ln_stats_v(src_ap):
            """bn_stats path -> (rstd, nmr) [128,1]; V + one scalar sqrt."""
            st6 = stat.tile([128, 6], F32)
            nc.vector.bn_stats(st6[:], src_ap)
            mv = stat.tile([128, 2], F32)
            nc.vector.bn_aggr(mv[:], st6[:])
            std = stat.tile([128, 1], F32)
            nc.scalar.activation(std[:], mv[:, 1:2], AF.Sqrt, bias=epst[:])
            rstd = stat.tile([128, 1], F32)
            nc.vector.reciprocal(rstd[:], std[:])
            nmr = stat.tile([128, 1], F32)
            nc.vector.tensor_scalar(
                nmr[:], mv[:, 0:1], rstd[:], -1.0, op0=AO.mult, op1=AO.mult)
            return rstd, nmr

        # persistent scan carries, one [128,1] per (b, p, comp)
        ginit = {}
        for b in range(BS):
            for p in range(2):
                for comp, src in ((0, q0r_d), (1, q0i_d)):
                    t = gip.tile([128, 1], F32, name="giq")
                    nc.sync.dma_start(t[:], src[b, p * 128:(p + 1) * 128])
                    ginit[(b, p, comp)] = t

        def emit_a(c, b):
            """Stage 1+2: load, LN1 (batched small-ops), leaky, transpose,
            B-projection into PSUM."""
            t0 = c * CT
            if True:
                # ---- stage 1: load, LN1, leaky, transpose ----
                zt = ztp.tile([128, 2 * CT], F32, name="zt")
                zsk = []
                for s in range(NSUB):
                    xt = xin.tile([128, D], F32)
                    nc.sync.dma_start(
                        xt[:], x_d[b, t0 + s * 128:t0 + (s + 1) * 128, :])
                    rstd, nmr = ln_stats_v(xt[:])
                    z = zskip.tile([128, D], F32)
                    nc.scalar.activation(
                        z[:], xt[:], AF.Identity, bias=nmr[:], scale=rstd[:])
                    if g1:
                        nc.vector.tensor_mul(z[:], z[:], gbt["g1"][:])
                        nc.vector.tensor_add(z[:], z[:], gbt["b1"][:])
                    zsk.append(z)
                    zl = zlp.tile([128, D], F32)
                    nc.scalar.activation(zl[:], z[:], AF.Prelu, alpha=SLOPE)
                    pt = ptr.tile([128, 256], F32, name="pt", tag="pt")
                    for k in range(2):
                        nc.tensor.transpose(
                            pt[:, k * 128:(k + 1) * 128],
                            zl[:, k * 128:(k + 1) * 128], ident[:])
                    for k in range(2):
                        copy_op(
                            "copy1",
                            zt[:, k * CT + s * 128:k * CT + (s + 1) * 128],
                            pt[:, k * 128:(k + 1) * 128])
                # ---- stage 2: B projection -> u2[p] = [u_re | u_im] PSUM ----
                u2 = []
                for p in range(2):
                    ut = pu.tile([128, 2 * CT], F32)
                    for comp, bt in ((0, brt), (1, bit)):
                        for k in range(2):
                            nc.tensor.matmul(
                                ut[:, comp * CT:(comp + 1) * CT],
                                bt[k][:, p * 128:(p + 1) * 128],
                                zt[:, k * CT:(k + 1) * CT],
                                start=(k == 0), stop=(k == 1))
                    u2.append(ut)
            return dict(zt=zt, zsk=zsk, u2=u2)

        def emit_b(c, b, st):
            """Stage 3+4: pre-rotation, scans, carry, post-rotation."""
            t0 = c * CT
            u2 = st["u2"]
            if True:
                # ---- stage 3: pre-rotation + scans + carry ----
                # both prerots first (frees the u2 PSUM ring for the next
                # iteration's B-projection as early as possible)
                wcs = []
                for p in range(2):
                    mc = big.tile([128, 2 * CT], F32)
                    eng("prerot").tensor_mul(mc[:], cos2[p][:], u2[p][:])
                    md = big.tile([128, 2 * CT], F32)
                    eng("prerot").tensor_mul(md[:], sin2[p][:], u2[p][:])
                    wcs.append((mc, md))
                gts = []
                for p in range(2):
                    if use_mask:
                        d0 = d0p.tile([128, CT], F32)
                        nc.sync.dma_start(
                            d0[:], d0_d[b, p * 128:(p + 1) * 128, t0:t0 + CT])
                        d0ap = d0[:]
                    else:
                        d0ap = rbc[p][:]
                    mc, md = wcs[p]
                    wr = wp.tile([128, CT], F32)
                    eng("wcomb").tensor_add(wr[:], mc[:, :CT], md[:, CT:])
                    wi = wp.tile([128, CT], F32)
                    eng("wcomb").tensor_sub(wi[:], mc[:, CT:], md[:, :CT])
                    gr = g2p.tile([128, CT], F32, name="gr")
                    eng("scan").tensor_tensor_scan(
                        gr[:], d0ap, wr[:], ginit[(b, p, 0)][:],
                        op0=AO.mult, op1=AO.add)
                    gi = g2p.tile([128, CT], F32, name="gi")
                    eng("scan").tensor_tensor_scan(
                        gi[:], d0ap, wi[:], ginit[(b, p, 1)][:],
                        op0=AO.mult, op1=AO.add)
                    gts.append((gr, gi))
                    if c + 1 < NCH:
                        grl = gr[:, CT - 1:CT]
                        gil = gi[:, CT - 1:CT]
                        t5 = gip.tile([128, 1], F32)
                        eng("carry").tensor_scalar_mul(t5[:], gil, esl[p][:])
                        ngr = gip.tile([128, 1], F32)
                        eng("carry").scalar_tensor_tensor(
                            ngr[:], grl, ecl[p][:], t5[:],
                            op0=AO.mult, op1=AO.subtract)
                        t6 = gip.tile([128, 1], F32)
                        eng("carry").tensor_scalar_mul(t6[:], grl, esl[p][:])
                        ngi = gip.tile([128, 1], F32)
                        eng("carry").scalar_tensor_tensor(
                            ngi[:], gil, ecl[p][:], t6[:],
                            op0=AO.mult, op1=AO.add)
                        ginit[(b, p, 0)], ginit[(b, p, 1)] = ngr, ngi
                # ---- stage 4: post-rotation -> h2[p] = [h_re | h_im] ----
                h2 = []
                for p in range(2):
                    gr, gi = gts[p]
                    cosj = cos2[p][:, :CT]
                    sinj = sin2[p][:, :CT]
                    ma = big.tile([128, 2 * CT], F32)
                    eng("postA").tensor_mul(ma[:, :CT], cosj, gr[:])
                    eng("postA").tensor_mul(ma[:, CT:], cosj, gi[:])
                    mb = big.tile([128, 2 * CT], F32)
                    eng("postB").tensor_mul(mb[:, :CT], sinj, gr[:])
                    eng("postB").tensor_mul(mb[:, CT:], sinj, gi[:])
                    ht = h2p.tile([128, 2 * CT], F32, name="h2")
                    eng("hcomb").tensor_sub(ht[:, :CT], ma[:, :CT], mb[:, CT:])
                    eng("hcomb").tensor_add(ht[:, CT:], ma[:, CT:], mb[:, :CT])
                    h2.append(ht)
            st["h2"] = h2
            return st

        def emit_c(c, b, st):
            """Stages 5-6: C/D projection, LN2, MLP, LN3, skip, store."""
            t0 = c * CT
            zt, h2, zsk = st["zt"], st["h2"], st["zsk"]
            if True:
                # ---- stage 5: C/D projection + LN2 + leaky + transpose ----
                y2t = y2tp.tile([128, 2 * CT], F32, name="y2t")
                yl2s = []
                for s in range(NSUB):
                    pt1 = pyx.tile([128, D], F32, name="pyx", tag="pyx")[:]
                    mms = []
                    for k in range(2):
                        mms.append((zt[:, k * CT + s * 128:k * CT + (s + 1) * 128],
                                    dts[k][:]))
                    for p in range(2):
                        mms.append((h2[p][:, s * 128:(s + 1) * 128], crt[p][:]))
                        mms.append((h2[p][:, CT + s * 128:CT + (s + 1) * 128],
                                    cin[p][:]))
                    for i, (lhs, rhs) in enumerate(mms):
                        nc.tensor.matmul(pt1, lhs, rhs, start=(i == 0),
                                         stop=(i == len(mms) - 1))
                    rstd, nmr = ln_stats_v(pt1)
                    yl2 = yl2p.tile([128, D], F32)
                    if g2:
                        nc.scalar.activation(
                            yl2[:], pt1, AF.Identity, bias=nmr[:], scale=rstd[:])
                        nc.vector.tensor_mul(yl2[:], yl2[:], gbt["g2"][:])
                        nc.vector.tensor_add(yl2[:], yl2[:], gbt["b2"][:])
                        nc.scalar.activation(yl2[:], yl2[:], AF.Prelu, alpha=SLOPE)
                    else:
                        nc.scalar.activation(
                            yl2[:], pt1, AF.Prelu, bias=nmr[:], scale=rstd[:],
                            alpha=SLOPE)
                    yl2s.append(yl2)
                for s in range(NSUB):
                    ptt = ptr.tile([128, 256], F32, name="pt", tag="pt")
                    for p in range(2):
                        nc.tensor.transpose(
                            ptt[:, p * 128:(p + 1) * 128],
                            yl2s[s][:, p * 128:(p + 1) * 128], ident[:])
                    for p in range(2):
                        copy_op(
                            "copy5",
                            y2t[:, p * CT + s * 128:p * CT + (s + 1) * 128],
                            ptt[:, p * 128:(p + 1) * 128])
                # ---- stage 6: MLP + LN3 + skip + store ----
                for s in range(NSUB):
                    pt3 = pyx.tile([128, D], F32, name="pyx", tag="pyx")[:]
                    for p in range(2):
                        nc.tensor.matmul(
                            pt3, y2t[:, p * CT + s * 128:p * CT + (s + 1) * 128],
                            mts[p][:], start=(p == 0), stop=(p == 1))
                    if use_bias:
                        nc.vector.tensor_add(pt3, pt3, gbt["mb"][:])
                    rstd, nmr = ln_stats_v(pt3)
                    yo = yop.tile([128, D], F32)
                    nc.scalar.activation(
                        yo[:], pt3, AF.Identity, bias=nmr[:], scale=rstd[:])
                    if g3:
                        nc.vector.tensor_mul(yo[:], yo[:], gbt["g3"][:])
                        nc.vector.tensor_add(yo[:], yo[:], gbt["b3"][:])
                    eng("skip").tensor_add(yo[:], yo[:], zsk[s][:])
                    nc.sync.dma_start(
                        out_d[b, t0 + s * 128:t0 + (s + 1) * 128, :], yo[:])

        # 3-phase software pipeline: per step k emit A(k), B(k-1), C(k-2)
        # so each engine queue interleaves three iterations and the PE
        # always has independent matmul work while the V/G rotation/scan
        # chains of older iterations complete.
        iters = [(c, b) for c in range(NCH) for b in range(BS)]
        n = len(iters)
        states = {}
        for k in range(n + 2):
            if 1 <= k and k - 1 < n:
                emit_b(*iters[k - 1], states[k - 1])
            if k < n:
                states[k] = emit_a(*iters[k])
            if 2 <= k:
                emit_c(*iters[k - 2], states.pop(k - 2))
    nc.compile()
    return nc


def _prep_host(inputs):
    """Host-side precompute: tables, folded weights, per-core input maps."""
    x = np.asarray(inputs["x"], np.float32)
    done = np.asarray(inputs["done"])
    h0r = np.asarray(inputs["h0_re"], np.float32)
    h0i = np.asarray(inputs["h0_im"], np.float32)
    nu = np.asarray(inputs["nu_log"], np.float64)
    th_log = np.asarray(inputs["theta_log"], np.float64)
    gl = np.asarray(inputs["gamma_log"], np.float64)

    r = np.exp(-np.exp(nu))                     # |lambda|, [N]
    theta = np.exp(th_log)                      # [N]
    gamma = np.exp(gl)

    j = np.arange(CT, dtype=np.float64)
    ang = theta[:, None] * j[None, :]           # [N, CT]
    cosj = np.cos(ang).astype(np.float32)
    sinj = np.sin(ang).astype(np.float32)
    cos2 = np.concatenate([cosj, cosj], axis=1)  # [N, 2CT]
    sin2 = np.concatenate([sinj, sinj], axis=1)
    rbc = np.repeat(r.astype(np.float32)[:, None], CT, axis=1)
    angL = theta * CT
    ecl = np.cos(angL).astype(np.float32)[:, None]
    esl = np.sin(angL).astype(np.float32)[:, None]

    # q0 = e^{i theta} * h0  per (b, n)
    c1, s1 = np.cos(theta), np.sin(theta)
    q0r = (c1[None, :] * h0r - s1[None, :] * h0i).astype(np.float32)
    q0i = (c1[None, :] * h0i + s1[None, :] * h0r).astype(np.float32)

    brt = np.ascontiguousarray(
        (np.asarray(inputs["B_re"], np.float64) * gamma[:, None]).T
    ).astype(np.float32)
    bit = np.ascontiguousarray(
        (np.asarray(inputs["B_im"], np.float64) * gamma[:, None]).T
    ).astype(np.float32)
    crt = np.ascontiguousarray(np.asarray(inputs["C_re"], np.float32).T)
    cin = np.ascontiguousarray(-np.asarray(inputs["C_im"], np.float32).T)
    dt = np.ascontiguousarray(np.asarray(inputs["D_mat"], np.float32).T)
    mt = np.ascontiguousarray(np.asarray(inputs["mlp_w"], np.float32).T)

    g1v = np.asarray(inputs["ln1_g"], np.float32)
    b1v = np.asarray(inputs["ln1_b"], np.float32)
    g2v = np.asarray(inputs["ln2_g"], np.float32)
    b2v = np.asarray(inputs["ln2_b"], np.float32)
    g3v = np.asarray(inputs["ln3_g"], np.float32)
    b3v = np.asarray(inputs["ln3_b"], np.float32)
    mbv = np.asarray(inputs["mlp_b"], np.float32)

    g1 = not (np.all(g1v == 1) and np.all(b1v == 0))
    g2 = not (np.all(g2v == 1) and np.all(b2v == 0))
    g3 = not (np.all(g3v == 1) and np.all(b3v == 0))
    use_bias = bool(np.any(mbv != 0))
    use_mask = bool(np.any(done))
    flags = (g1, g2, g3, use_bias, use_mask)

    shared = dict(cos2=cos2, sin2=sin2, rbc=rbc, ecl=ecl, esl=esl,
                  BrT=brt, BiT=bit, CrT=crt, CiTn=cin, DT=dt, MT=mt)

    def bc(v):
        return np.ascontiguousarray(np.broadcast_to(v[None, :], (128, D))
                                    ).astype(np.float32)
    if g1:
        shared["g1bc"], shared["b1bc"] = bc(g1v), bc(b1v)
    if g2:
        shared["g2bc"], shared["b2bc"] = bc(g2v), bc(b2v)
    if g3:
        shared["g3bc"], shared["b3bc"] = bc(g3v), bc(b3v)
    if use_bias:
        shared["mbbc"] = bc(mbv)

    in_maps = []
    for core in range(NCORES):
        sl = slice(core * BS, (core + 1) * BS)
        m = dict(shared)
        m["x"] = np.ascontiguousarray(x[sl])
        m["q0r"] = np.ascontiguousarray(q0r[sl])
        m["q0i"] = np.ascontiguousarray(q0i[sl])
        if use_mask:
            mask = 1.0 - done[sl].astype(np.float32)       # [BS, T]
            d0 = (rbc[None, :, 0:1] * mask[:, None, :])    # [BS, N, T]
            m["d0tab"] = np.ascontiguousarray(d0.astype(np.float32))
        in_maps.append(m)
    return flags, in_maps


def _get_program(flags):
    if flags not in _PROG_CACHE:
        _PROG_CACHE[flags] = _build_program(flags)
    return _PROG_CACHE[flags]


def run(inputs, trace=False, **kw):
    flags, in_maps = _prep_host(inputs)
    nc = _get_program(flags)
    res = run_bass_kernel_spmd(nc, in_maps, list(range(NCORES)),
                               trace=trace, **kw)
    out = np.concatenate([res.results[i]["out"] for i in range(NCORES)], axis=0)
    return out, res


def kernel(**inputs):
    out, _ = run(inputs, trace=False)
    return out
